# revision 1
# baseline (speedup 1.0000x reference)
"""Trainium2 Bass kernel: 4-head transformer core (attention + residual + LayerNorm).

Reference computation (per batch b of 4, seq 2048, d_model 1024, 4 heads x 256):
    qkv = x @ qkv_w.T + qkv_b ; q,k,v per head
    attn = softmax(q k^T / 16) ; out = attn v
    y = x + out @ wo_w.T + wo_b ; layernorm(y) * gamma + beta

Sharding: pure data parallel over (batch, seq-half) -> 8 cores, no collectives.
Each core handles 1024 query tokens of one batch; K/V are computed for the
full 2048 tokens of that batch (duplicated across the 2 cores of a batch).
Host passes x pre-transposed (d-major) and rotated so the core's local tokens
are always columns [0, 1024) -- one SPMD program serves all cores. Attention
is permutation-invariant over key/value positions, so the rotation does not
change the result.

On-chip layouts (partition dim first):
    qT  [128, 8, 1024]  Q^T  feature-major (feat-tile = 2h+echunk)
    kT  [128, 8, 2048]  K^T  feature-major
    v   [128, 16, 1024] V    token-major (kv-token-tile, feat)
    scores^T in PSUM [ktok, qtok]; exp on ACT; denominator via ones-matmul;
    out^T [feat, q] scaled by 1/denom broadcast; wo matmul emits y token-major;
    LayerNorm along the free dim via bn_stats/bn_aggr.

All matmuls in bf16 (fp32 PSUM accumulate); softmax/residual/layernorm in fp32.
"""

import os

import ml_dtypes
import numpy as np

P = 128
B, S, D = 4, 2048, 1024
H = 4
HD = D // H  # 256
SL = S // 2  # local query tokens per core
DC = D // P  # 8 d-chunks
QT_TILES = SL // P  # 8
KT_TILES = S // P  # 16
NQ = SL // 512  # 2 q-chunks of 512
EPS = 1e-5
NCORES = 8

_BF16 = ml_dtypes.bfloat16

_CACHE = {}


def _install_drain_patch():
    """walrus CoreV3 in this container accepts at most one sem wait per SP
    CTRL instruction, but Tile's kernel-tail drain carries one wait per
    outstanding logical proc.  Redistribute them onto single-wait no-ops."""
    import concourse.tile as _tile
    from concourse import mybir
    from concourse.vector_clock import ScopedClock

    if getattr(_tile.TileContext, "_drain_patch_installed", False):
        return

    def _drain_and_barrier(self, tick_clock, wait_clock):
        nc = self.nc
        drain_inst = nc.sync.drain()
        wait_clock.add_sem_waits(
            drain_inst.ins, ScopedClock({None: tick_clock.global_clock})
        )
        si = drain_inst.ins.sync_info
        if si is not None and len(si.on_wait) > 1:
            waits = list(si.on_wait)
            drain_inst.ins.sync_info = mybir.SyncInfo(
                on_wait=[], on_update=list(si.on_update)
            )
            for w in waits:
                nop = nc.sync.nop(nofuse=True, hint="drain_wait_split")
                nop.ins.sync_info = mybir.SyncInfo(on_wait=[w], on_update=[])

        nc.all_engine_barrier()
        assert self.sems is not None
        popped = nc._tile_sem_poison_stack.pop()
        assert popped is self._sem_poison
        nc.clear_and_free_semaphores(list(self.sems.allocated().values()))
        nc.all_engine_barrier()

    _tile.TileContext._drain_and_barrier = _drain_and_barrier
    _tile.TileContext._drain_patch_installed = True


def _split_excess_waits(nc):
    """This walrus build accepts at most one sem wait per instruction (two for
    EventSemaphore), but Tile attaches one wait per outstanding proc.  Move
    the excess waits onto same-engine no-ops inserted just before each
    over-subscribed instruction (same-engine program order makes the waits
    complete before the instruction issues)."""
    from concourse import mybir

    n_split = 0
    for f in nc.m.functions:
        for b in f.blocks:
            insts = b.instructions
            new_list = []
            changed = False
            for inst in insts:
                si = inst.sync_info
                cap = 2 if isinstance(inst, mybir.InstEventSemaphore) else 1
                if si is not None and len(si.on_wait) > cap:
                    waits = list(si.on_wait)
                    for k, w in enumerate(waits[:-cap]):
                        nop = mybir.InstNoOp(name=f"{inst.name}-ws{k}")
                        nop.engine = inst.engine
                        nop.bass_nofuse = True
                        nop.sync_info = mybir.SyncInfo(on_wait=[w], on_update=[])
                        new_list.append(nop)
                        n_split += 1
                    inst.sync_info = mybir.SyncInfo(
                        on_wait=waits[-cap:], on_update=list(si.on_update)
                    )
                    changed = True
                new_list.append(inst)
            if changed:
                b.instructions = new_list
    return n_split


def _build(ZB=False):
    """ZB: specialize for qkv_b == 0, wo_b folded on host, gamma == 1, beta == 0."""
    import concourse.bass as bass
    import concourse.tile as tile
    from concourse import mybir

    _install_drain_patch()

    f32 = mybir.dt.float32
    bf16 = mybir.dt.bfloat16
    AF = mybir.ActivationFunctionType
    ALU = mybir.AluOpType

    nc = bass.Bass()

    xT_d = nc.dram_tensor("xT", [D, S], bf16, kind="ExternalInput")
    wqkvT_d = nc.dram_tensor("wqkvT", [D, 3 * D], bf16, kind="ExternalInput")
    woT_d = nc.dram_tensor("woT", [D, D], bf16, kind="ExternalInput")
    xb_d = nc.dram_tensor("xb", [SL, D], f32, kind="ExternalInput")
    qkvb_d = nc.dram_tensor("qkvb", [P, 24], f32, kind="ExternalInput")
    vb_d = nc.dram_tensor("vb", [P, D], f32, kind="ExternalInput")
    gamma_d = nc.dram_tensor("gamma", [P, D], f32, kind="ExternalInput")
    beta_d = nc.dram_tensor("beta", [P, D], f32, kind="ExternalInput")
    y_d = nc.dram_tensor("y", [SL, D], f32, kind="ExternalOutput")

    def bcast_ap(handle):
        ap = handle[:]
        return bass.AP(tensor=ap.tensor, offset=ap.offset, ap=[[0, P]] + list(ap.ap))

    with tile.TileContext(nc) as tc:
        with tc.tile_pool(name="persist", bufs=1) as pp:
            qT = pp.tile([P, DC, SL], bf16, tag="qT")
            kT = pp.tile([P, DC, S], bf16, tag="kT")
            v = pp.tile([P, KT_TILES, D], bf16, tag="v")
            outT = pp.tile([P, DC, SL], bf16, tag="outT")
            woT = pp.tile([P, DC, D], bf16, tag="woT")
            if not ZB:
                gamma_bc = pp.tile([P, D], f32, tag="gamma_bc")
                beta_bc = pp.tile([P, D], f32, tag="beta_bc")
                vb_bc = pp.tile([P, D], f32, tag="vb_bc")
                qkvb = pp.tile([P, 24], f32, tag="qkvb")
            ones_k = pp.tile([P, 1], bf16, tag="ones_k")
            ones_m = pp.tile([1, P], bf16, tag="ones_m")
            eps_t = pp.tile([P, 1], f32, tag="eps")

            nc.vector.memset(ones_k, 1.0)
            nc.vector.memset(ones_m, 1.0)
            nc.vector.memset(eps_t, EPS)

            # ---- Stage A: QKV projections ----
            with (
                tc.tile_pool(name="stA", bufs=1) as pa,
                tc.tile_pool(name="stA_w", bufs=3) as paw,
                tc.tile_pool(name="psA", bufs=6, space="PSUM") as psA,
            ):
                xT = pa.tile([P, DC, S], bf16, tag="xT")
                wv = pa.tile([P, DC, D], bf16, tag="wv")
                wqkvT_r = wqkvT_d[:].rearrange("(c p) f -> p c f", p=P)
                xT_r = xT_d[:].rearrange("(c p) t -> p c t", p=P)

                # Head-latency critical: the very first matmul needs only
                # wqk[m=0] and xT tokens [0:512).  Spread the input loads
                # across the SP/ACT/DVE DMA queues so they run in parallel
                # and the first weight tile arrives first.
                def load_wqk(m, eng):
                    t = paw.tile([P, DC, P], bf16, tag="wqk")
                    eng.dma_start(out=t, in_=wqkvT_r[:, :, m * P : (m + 1) * P])
                    return t

                wqk_tiles = {0: load_wqk(0, nc.sync)}
                for tch in range(4):
                    nc.scalar.dma_start(
                        out=xT[:, :, tch * 512 : (tch + 1) * 512],
                        in_=xT_r[:, :, tch * 512 : (tch + 1) * 512],
                    )
                wqk_tiles[1] = load_wqk(1, nc.sync)
                if not ZB:
                    nc.gpsimd.dma_start(out=qkvb, in_=qkvb_d[:])
                    nc.gpsimd.dma_start(out=vb_bc, in_=vb_d[:])
                nc.scalar.dma_start(out=wv, in_=wqkvT_r[:, :, 2 * D : 3 * D])

                # Q (local tokens) and K (all tokens), feature-major
                for m in range(16):
                    if m in wqk_tiles:
                        wqk = wqk_tiles.pop(m)
                    else:
                        wqk = load_wqk(m, nc.sync)
                    ntok = SL if m < 8 else S
                    for qc in range(ntok // 512):
                        ps = psA.tile([P, 512], f32, tag="psA")
                        for dc in range(DC):
                            nc.tensor.matmul(
                                ps,
                                lhsT=wqk[:, dc, :],
                                rhs=xT[:, dc, qc * 512 : (qc + 1) * 512],
                                start=(dc == 0),
                                stop=(dc == DC - 1),
                            )
                        if m < 8:
                            dst = qT[:, m, qc * 512 : (qc + 1) * 512]
                        else:
                            dst = kT[:, m - 8, qc * 512 : (qc + 1) * 512]
                        nc.scalar.activation(
                            out=dst,
                            in_=ps,
                            func=AF.Identity,
                            bias=0.0 if ZB else qkvb[:, m : m + 1],
                            scale=1.0,
                        )

                # V, token-major
                for vt in range(KT_TILES):
                    for oc in range(2):
                        ps = psA.tile([P, 512], f32, tag="psA")
                        for dc in range(DC):
                            nc.tensor.matmul(
                                ps,
                                lhsT=xT[:, dc, vt * P : (vt + 1) * P],
                                rhs=wv[:, dc, oc * 512 : (oc + 1) * 512],
                                start=(dc == 0),
                                stop=(dc == DC - 1),
                            )
                        if ZB:
                            nc.vector.tensor_copy(
                                out=v[:, vt, oc * 512 : (oc + 1) * 512], in_=ps
                            )
                        else:
                            nc.vector.tensor_add(
                                out=v[:, vt, oc * 512 : (oc + 1) * 512],
                                in0=ps,
                                in1=vb_bc[:, oc * 512 : (oc + 1) * 512],
                            )

            # ---- Stage B/C: attention per (head, q-chunk) ----
            with (
                tc.tile_pool(name="es_pool", bufs=2) as pes,
                tc.tile_pool(name="bc_pool", bufs=2) as pbc,
                tc.tile_pool(name="ps_sc", bufs=2, space="PSUM") as ps_sc,
                tc.tile_pool(name="ps_out", bufs=4, space="PSUM") as ps_out,
                tc.tile_pool(name="ps_den", bufs=1, space="PSUM") as ps_den,
                tc.tile_pool(name="ps_bc", bufs=1, space="PSUM") as ps_bcp,
            ):
                # The softmax denominator + 1/den broadcast + out^T scaling for
                # unit u are emitted in the middle of unit u+1 (software
                # pipelining) so the reciprocal's latency never stalls the PE
                # and single PSUM banks suffice for den/bcast.

                def act_reciprocal(out, in_):
                    # ACT LUT reciprocal: blocked by bass for general use
                    # (LUT accuracy), but softmax denominators are smooth
                    # positives and the attention path tolerates ~1e-3.
                    eng = nc.scalar
                    ins_ = [
                        eng.lower_ap(in_),
                        mybir.ImmediateValue(dtype=mybir.dt.float32, value=0.0),
                        mybir.ImmediateValue(dtype=mybir.dt.float32, value=1.0),
                        mybir.ImmediateValue(dtype=mybir.dt.float32, value=0.0),
                    ]
                    return eng.add_instruction(
                        mybir.InstActivation(
                            name=nc.get_next_instruction_name(),
                            func=AF.Reciprocal,
                            ins=ins_,
                            outs=[eng.lower_ap(out)],
                        )
                    )

                def finalize_den(p):
                    es_p, ops_p, hqc = p
                    dps = ps_den.tile([1, 512], f32, tag="ps_den")
                    for j in range(KT_TILES):
                        nc.tensor.matmul(
                            dps,
                            lhsT=ones_k,
                            rhs=es_p[:, j, :],
                            start=(j == 0),
                            stop=(j == KT_TILES - 1),
                        )
                    den_bf = pbc.tile([1, 512], bf16, tag="den_bf")
                    nc.vector.tensor_copy(out=den_bf, in_=dps)
                    return (ops_p, hqc, den_bf)

                def finalize_bps(p2):
                    ops_p, hqc, den_bf = p2
                    bps = ps_bcp.tile([P, 512], f32, tag="ps_bc")
                    nc.tensor.matmul(
                        bps, lhsT=ones_m, rhs=den_bf, start=True, stop=True
                    )
                    return (ops_p, hqc, bps)

                def finalize_scale(p3):
                    ops_p, (h_p, qc_p), bps = p3
                    qsl_p = slice(qc_p * 512, (qc_p + 1) * 512)
                    bc = pbc.tile([P, 512], f32, tag="bc")
                    nc.vector.reciprocal(out=bc, in_=bps)
                    for f in range(2):
                        nc.vector.tensor_mul(
                            out=outT[:, 2 * h_p + f, qsl_p], in0=ops_p[f], in1=bc
                        )

                pend = None
                pend2 = None
                pend3 = None
                for h in range(H):
                    for qc in range(NQ):
                        qsl = slice(qc * 512, (qc + 1) * 512)
                        es = pes.tile([P, KT_TILES, 512], bf16, tag="es")
                        op0 = ps_out.tile([P, 512], f32, tag="ps_out")
                        op1 = ps_out.tile([P, 512], f32, tag="ps_out")
                        ops = [op0, op1]
                        for j in range(KT_TILES):
                            sps = ps_sc.tile([P, 512], f32, tag="ps_sc")
                            for ec in range(2):
                                nc.tensor.matmul(
                                    sps,
                                    lhsT=kT[:, 2 * h + ec, j * P : (j + 1) * P],
                                    rhs=qT[:, 2 * h + ec, qsl],
                                    start=(ec == 0),
                                    stop=(ec == 1),
                                )
                            nc.scalar.activation(
                                out=es[:, j, :], in_=sps, func=AF.Exp, scale=1.0 / 16.0
                            )
                            if j == 4 and pend is not None:
                                pend2 = finalize_den(pend)
                                pend = None
                            if j == 8 and pend2 is not None:
                                pend3 = finalize_bps(pend2)
                                pend2 = None
                            if j == 13 and pend3 is not None:
                                finalize_scale(pend3)
                                pend3 = None
                            for f in range(2):
                                nc.tensor.matmul(
                                    ops[f],
                                    lhsT=v[
                                        :, j, h * HD + f * P : h * HD + (f + 1) * P
                                    ],
                                    rhs=es[:, j, :],
                                    start=(j == 0),
                                    stop=(j == KT_TILES - 1),
                                )
                        pend = (es, ops, (h, qc))
                finalize_scale(finalize_bps(finalize_den(pend)))

            # ---- Stage D: wo projection + residual + LayerNorm ----
            with (
                tc.tile_pool(name="stD", bufs=3) as pd,
                tc.tile_pool(name="stD_y", bufs=2) as pdy,
                tc.tile_pool(name="stD_st", bufs=4) as pst,
                tc.tile_pool(name="ps_wo", bufs=4, space="PSUM") as ps_wo,
            ):
                nc.sync.dma_start(
                    out=woT, in_=woT_d[:].rearrange("(c p) o -> p c o", p=P)
                )
                if not ZB:
                    nc.sync.dma_start(out=gamma_bc, in_=gamma_d[:])
                    nc.sync.dma_start(out=beta_bc, in_=beta_d[:])
                for qt in range(QT_TILES):
                    y_t = pdy.tile([P, D], f32, tag="y")
                    for oc in range(2):
                        wps = ps_wo.tile([P, 512], f32, tag="ps_wo")
                        for dc in range(DC):
                            nc.tensor.matmul(
                                wps,
                                lhsT=outT[:, dc, qt * P : (qt + 1) * P],
                                rhs=woT[:, dc, oc * 512 : (oc + 1) * 512],
                                start=(dc == 0),
                                stop=(dc == DC - 1),
                            )
                        nc.scalar.activation(
                            out=y_t[:, oc * 512 : (oc + 1) * 512],
                            in_=wps,
                            func=AF.Identity,
                            bias=0.0,
                            scale=1.0,
                        )
                    # residual add in the DMA compute engine: y_t += xb
                    nc.gpsimd.dma_start(
                        out=y_t,
                        in_=xb_d[qt * P : (qt + 1) * P, :],
                        accum_op=ALU.add,
                    )
                    stats = pst.tile([P, 2, 6], f32, tag="stats")
                    for sg in range(2):
                        nc.vector.bn_stats(
                            out=stats[:, sg, :], in_=y_t[:, sg * 512 : (sg + 1) * 512]
                        )
                    mv = pst.tile([P, 2], f32, tag="mv")
                    nc.vector.bn_aggr(out=mv, in_=stats)
                    std = pst.tile([P, 1], f32, tag="std")
                    nc.scalar.activation(
                        out=std, in_=mv[:, 1:2], func=AF.Sqrt, bias=eps_t, scale=1.0
                    )
                    rstd = pst.tile([P, 1], f32, tag="rstd")
                    nc.vector.reciprocal(out=rstd, in_=std)
                    nc.vector.tensor_scalar(
                        out=y_t,
                        in0=y_t,
                        scalar1=mv[:, 0:1],
                        scalar2=rstd,
                        op0=ALU.subtract,
                        op1=ALU.mult,
                    )
                    if not ZB:
                        nc.vector.tensor_mul(out=y_t, in0=y_t, in1=gamma_bc)
                        nc.vector.tensor_add(out=y_t, in0=y_t, in1=beta_bc)
                    nc.sync.dma_start(out=y_d[qt * P : (qt + 1) * P, :], in_=y_t)

    _split_excess_waits(nc)
    return nc


def _get_nc(ZB):
    key = ("nc", ZB)
    if key not in _CACHE:
        _CACHE[key] = _build(ZB)
    return _CACHE[key]


def kernel(x, qkv_w, qkv_b, wo_w, wo_b, gamma, beta):
    from concourse.bass_utils import run_bass_kernel_spmd

    x = np.asarray(x, dtype=np.float32)
    qkv_w = np.asarray(qkv_w, dtype=np.float32)
    qkv_b = np.asarray(qkv_b, dtype=np.float32)
    wo_w = np.asarray(wo_w, dtype=np.float32)
    wo_b = np.asarray(wo_b, dtype=np.float32)
    gamma = np.asarray(gamma, dtype=np.float32)
    beta = np.asarray(beta, dtype=np.float32)

    wqkvT = np.ascontiguousarray(qkv_w.T).astype(_BF16)
    woT = np.ascontiguousarray(wo_w.T).astype(_BF16)
    qkvb2d = np.ascontiguousarray(qkv_b.reshape(24, P).T)
    vb = np.ascontiguousarray(np.broadcast_to(qkv_b[2 * D :], (P, D))).astype(np.float32)
    gamma_r = np.ascontiguousarray(np.broadcast_to(gamma, (P, D))).astype(np.float32)
    beta_r = np.ascontiguousarray(np.broadcast_to(beta, (P, D))).astype(np.float32)

    in_maps = []
    for c in range(NCORES):
        b, t = divmod(c, 2)
        loc = x[b, t * SL : (t + 1) * SL]  # [SL, D]
        oth = x[b, (1 - t) * SL : (2 - t) * SL]
        xT = np.concatenate([loc.T, oth.T], axis=1).astype(_BF16)  # [D, S]
        xb = (loc + wo_b[None, :]).astype(np.float32)
        in_maps.append(
            {
                "xT": np.ascontiguousarray(xT),
                "wqkvT": wqkvT,
                "woT": woT,
                "xb": xb,
                "qkvb": qkvb2d,
                "vb": vb,
                "gamma": gamma_r,
                "beta": beta_r,
            }
        )

    zb = (
        not qkv_b.any()
        and bool(np.all(gamma == 1.0))
        and not beta.any()
    )
    trace = os.environ.get("BASS_KERNEL_TRACE") == "1"
    res = run_bass_kernel_spmd(
        _get_nc(zb),
        in_maps,
        list(range(NCORES)),
        trace=trace,
        tmpdir=os.environ.get("BASS_KERNEL_TRACE_DIR") or None,
    )
    _CACHE["last_result"] = res

    out = np.empty((B, S, D), dtype=np.float32)
    for c in range(NCORES):
        b, t = divmod(c, 2)
        out[b, t * SL : (t + 1) * SL] = res.results[c]["y"]
    return out



# revision 11
# speedup vs baseline: 1.4372x; 1.4372x over previous
"""Trainium2 Bass kernel: 4-head transformer core (attention + residual + LayerNorm).

Reference computation (per batch b of 4, seq 2048, d_model 1024, 4 heads x 256):
    qkv = x @ qkv_w.T + qkv_b ; q,k,v per head
    attn = softmax(q k^T / 16) ; out = attn v
    y = x + out @ wo_w.T + wo_b ; layernorm(y) * gamma + beta

Sharding: pure data parallel over (batch, seq-half) -> 8 cores, no collectives.
Each core handles 1024 query tokens of one batch; K/V are computed for the
full 2048 tokens of that batch (duplicated across the 2 cores of a batch).
Host passes x pre-transposed (d-major) and rotated so the core's local tokens
are always columns [0, 1024) -- one SPMD program serves all cores. Attention
is permutation-invariant over key/value positions, so the rotation does not
change the result.

All matmuls run in fp8e4 (e4m3) with DoubleRow perf mode: each instruction
contracts K=256 (two 128-partition subtiles packed in the free dim of both
operands) at the same per-instruction cost as one bf16 K=128 matmul -- 2x
the effective PE throughput.  The attention output is diluted ~25x by the
f32 residual before LayerNorm, so fp8 quantization of x/q/k/v/es/wo keeps
the final relative error ~1e-3, well inside the 2e-2 gate.

On-chip layouts (partition dim first):
    qT  [128, 8, 1024]  Q^T  feature-major (feat-tile = 2h+echunk)  fp8
    kT  [128, 8, 2048]  K^T  feature-major                          fp8
    v   [128, 16, 1024] V    token-major (kv-token-tile, feat)      fp8
    scores^T in PSUM [ktok, qtok]; exp on ACT -> es fp8; denominator via
    DoubleRow ones-matmul; 1/den on DVE pre-broadcast; broadcast via
    ones-matmul; outT [feat, q] = av_psum * bc on DVE (fp8 out);
    wo matmul emits y token-major f32; residual add via DMA-accum;
    LayerNorm along the free dim via bn_stats/bn_aggr.
"""

import os

import ml_dtypes
import numpy as np

P = 128
B, S, D = 4, 2048, 1024
H = 4
HD = D // H  # 256
SL = S // 2  # local query tokens per core
DC = D // P  # 8 d-chunks
QT_TILES = SL // P  # 8
KT_TILES = S // P  # 16
NQ = SL // 512  # 2 q-chunks of 512
EPS = 1e-5
NCORES = 8

_BF16 = ml_dtypes.bfloat16
_FP8 = ml_dtypes.float8_e4m3fn

_CACHE = {}


def _install_drain_patch():
    """walrus CoreV3 in this container accepts at most one sem wait per SP
    CTRL instruction, but Tile's kernel-tail drain carries one wait per
    outstanding logical proc.  Redistribute them onto single-wait no-ops."""
    import concourse.tile as _tile
    from concourse import mybir
    from concourse.vector_clock import ScopedClock

    if getattr(_tile.TileContext, "_drain_patch_installed", False):
        return

    def _drain_and_barrier(self, tick_clock, wait_clock):
        nc = self.nc
        drain_inst = nc.sync.drain()
        wait_clock.add_sem_waits(
            drain_inst.ins, ScopedClock({None: tick_clock.global_clock})
        )
        si = drain_inst.ins.sync_info
        if si is not None and len(si.on_wait) > 1:
            waits = list(si.on_wait)
            drain_inst.ins.sync_info = mybir.SyncInfo(
                on_wait=[], on_update=list(si.on_update)
            )
            for w in waits:
                nop = nc.sync.nop(nofuse=True, hint="drain_wait_split")
                nop.ins.sync_info = mybir.SyncInfo(on_wait=[w], on_update=[])

        nc.all_engine_barrier()
        assert self.sems is not None
        popped = nc._tile_sem_poison_stack.pop()
        assert popped is self._sem_poison
        nc.clear_and_free_semaphores(list(self.sems.allocated().values()))
        nc.all_engine_barrier()

    _tile.TileContext._drain_and_barrier = _drain_and_barrier
    _tile.TileContext._drain_patch_installed = True


def _split_excess_waits(nc):
    """This walrus build accepts at most one sem wait per instruction (two for
    EventSemaphore), but Tile attaches one wait per outstanding proc.  Move
    the excess waits onto same-engine no-ops inserted just before each
    over-subscribed instruction (same-engine program order makes the waits
    complete before the instruction issues)."""
    from concourse import mybir

    n_split = 0
    for f in nc.m.functions:
        for b in f.blocks:
            insts = b.instructions
            new_list = []
            changed = False
            for inst in insts:
                si = inst.sync_info
                cap = 2 if isinstance(inst, mybir.InstEventSemaphore) else 1
                if si is not None and len(si.on_wait) > cap:
                    waits = list(si.on_wait)
                    for k, w in enumerate(waits[:-cap]):
                        nop = mybir.InstNoOp(name=f"{inst.name}-ws{k}")
                        nop.engine = inst.engine
                        nop.bass_nofuse = True
                        nop.sync_info = mybir.SyncInfo(on_wait=[w], on_update=[])
                        new_list.append(nop)
                        n_split += 1
                    inst.sync_info = mybir.SyncInfo(
                        on_wait=waits[-cap:], on_update=list(si.on_update)
                    )
                    changed = True
                new_list.append(inst)
            if changed:
                b.instructions = new_list
    return n_split


def _build(ZB=False):
    """ZB: specialize for qkv_b == 0, wo_b folded on host, gamma == 1, beta == 0."""
    import concourse.bass as bass
    import concourse.tile as tile
    from concourse import mybir

    _install_drain_patch()

    f32 = mybir.dt.float32
    bf16 = mybir.dt.bfloat16
    fp8 = mybir.dt.float8e4
    AF = mybir.ActivationFunctionType
    ALU = mybir.AluOpType
    DR = mybir.MatmulPerfMode.DoubleRow

    nc = bass.Bass()

    xT_d = nc.dram_tensor("xT", [D, S], fp8, kind="ExternalInput")
    wqkvT_d = nc.dram_tensor("wqkvT", [D, 3 * D], fp8, kind="ExternalInput")
    woT_d = nc.dram_tensor("woT", [D, D], fp8, kind="ExternalInput")
    xb_d = nc.dram_tensor("xb", [SL, D], f32, kind="ExternalInput")
    qkvb_d = nc.dram_tensor("qkvb", [P, 24], f32, kind="ExternalInput")
    vb_d = nc.dram_tensor("vb", [P, D], f32, kind="ExternalInput")
    gamma_d = nc.dram_tensor("gamma", [P, D], f32, kind="ExternalInput")
    beta_d = nc.dram_tensor("beta", [P, D], f32, kind="ExternalInput")
    y_d = nc.dram_tensor("y", [SL, D], f32, kind="ExternalOutput")
    DBG = os.environ.get("KDBG") == "1"
    if DBG:
        dbg_qT = nc.dram_tensor("dbg_qT", [P, DC, SL], fp8, kind="ExternalOutput")
        dbg_kT = nc.dram_tensor("dbg_kT", [P, DC, S], fp8, kind="ExternalOutput")
        dbg_v = nc.dram_tensor("dbg_v", [P, KT_TILES, D], fp8, kind="ExternalOutput")
        dbg_es = nc.dram_tensor("dbg_es", [P, KT_TILES, 512], fp8, kind="ExternalOutput")
        dbg_bc = nc.dram_tensor("dbg_bc", [P, 512], f32, kind="ExternalOutput")
        dbg_outT = nc.dram_tensor("dbg_outT", [P, DC, SL], fp8, kind="ExternalOutput")

    with (
        tile.TileContext(nc) as tc,
        nc.allow_low_precision(reason="fp8 attention path, tolerance 2e-2"),
    ):
        with tc.tile_pool(name="persist", bufs=1) as pp:
            qT = pp.tile([P, DC, SL], fp8, tag="qT")
            kT = pp.tile([P, DC, S], fp8, tag="kT")
            v = pp.tile([P, KT_TILES, D], fp8, tag="v")
            outT = pp.tile([P, DC, SL], fp8, tag="outT")
            woT = pp.tile([P, DC, D], fp8, tag="woT")
            if not ZB:
                gamma_bc = pp.tile([P, D], f32, tag="gamma_bc")
                beta_bc = pp.tile([P, D], f32, tag="beta_bc")
                vb_bc = pp.tile([P, D], f32, tag="vb_bc")
                qkvb = pp.tile([P, 24], f32, tag="qkvb")
            ones_k2 = pp.tile([P, 2, P], fp8, tag="ones_k2")
            eps_t = pp.tile([P, 1], f32, tag="eps")
            nln32 = pp.tile([P, 1], f32, tag="nln32")

            nc.vector.memset(ones_k2, 1.0)
            nc.vector.memset(eps_t, EPS)
            nc.vector.memset(nln32, -3.4657359027997265)

            # ---- Stage A: QKV projections ----
            with (
                tc.tile_pool(name="stA", bufs=1) as pa,
                tc.tile_pool(name="stA_w", bufs=3) as paw,
                tc.tile_pool(name="psA", bufs=6, space="PSUM") as psA,
            ):
                xT = pa.tile([P, DC, S], fp8, tag="xT")
                wv = pa.tile([P, DC, D], fp8, tag="wv")
                wqkvT_r = wqkvT_d[:].rearrange("(c p) f -> p c f", p=P)
                xT_r = xT_d[:].rearrange("(c p) t -> p c t", p=P)

                # Head-latency critical: the very first matmul needs only
                # wqk[m=0] and xT tokens [0:512).  Spread the input loads
                # across the SP/ACT/DVE DMA queues so they run in parallel
                # and the first weight tile arrives first.
                def load_wqk(m, eng):
                    t = paw.tile([P, DC, P], fp8, tag="wqk")
                    eng.dma_start(out=t, in_=wqkvT_r[:, :, m * P : (m + 1) * P])
                    return t

                wqk_tiles = {0: load_wqk(0, nc.sync)}
                for tch in range(4):
                    nc.scalar.dma_start(
                        out=xT[:, :, tch * 512 : (tch + 1) * 512],
                        in_=xT_r[:, :, tch * 512 : (tch + 1) * 512],
                    )
                wqk_tiles[1] = load_wqk(1, nc.sync)
                if not ZB:
                    nc.gpsimd.dma_start(out=qkvb, in_=qkvb_d[:])
                    nc.gpsimd.dma_start(out=vb_bc, in_=vb_d[:])
                nc.scalar.dma_start(out=wv, in_=wqkvT_r[:, :, 2 * D : 3 * D])

                # Q (local tokens) and K (all tokens), feature-major.
                # Alternate the PSUM->SBUF fp8 cast between ACT and DVE to
                # balance engine load.
                for m in range(16):
                    if m in wqk_tiles:
                        wqk = wqk_tiles.pop(m)
                    else:
                        wqk = load_wqk(m, nc.sync)
                    ntok = SL if m < 8 else S
                    for qc in range(ntok // 512):
                        ps = psA.tile([P, 512], f32, tag="psA")
                        for dcp in range(DC // 2):
                            nc.tensor.matmul(
                                ps,
                                lhsT=wqk[:, 2 * dcp : 2 * dcp + 2, :],
                                rhs=xT[:, 2 * dcp : 2 * dcp + 2, qc * 512 : (qc + 1) * 512],
                                start=(dcp == 0),
                                stop=(dcp == DC // 2 - 1),
                                perf_mode=DR,
                            )
                        if m < 8:
                            dst = qT[:, m, qc * 512 : (qc + 1) * 512]
                        else:
                            dst = kT[:, m - 8, qc * 512 : (qc + 1) * 512]
                        if ZB:
                            if qc % 2 == 0:
                                nc.vector.tensor_copy(out=dst, in_=ps)
                            else:
                                nc.scalar.activation(
                                    out=dst, in_=ps, func=AF.Identity, bias=0.0, scale=1.0
                                )
                        else:
                            nc.scalar.activation(
                                out=dst,
                                in_=ps,
                                func=AF.Identity,
                                bias=qkvb[:, m : m + 1],
                                scale=1.0,
                            )

                # V, token-major
                for vt in range(KT_TILES):
                    for oc in range(2):
                        ps = psA.tile([P, 512], f32, tag="psA")
                        for dcp in range(DC // 2):
                            nc.tensor.matmul(
                                ps,
                                lhsT=xT[:, 2 * dcp : 2 * dcp + 2, vt * P : (vt + 1) * P],
                                rhs=wv[:, 2 * dcp : 2 * dcp + 2, oc * 512 : (oc + 1) * 512],
                                start=(dcp == 0),
                                stop=(dcp == DC // 2 - 1),
                                perf_mode=DR,
                            )
                        dst = v[:, vt, oc * 512 : (oc + 1) * 512]
                        if ZB:
                            if vt % 2 == 0:
                                nc.vector.tensor_copy(out=dst, in_=ps)
                            else:
                                nc.scalar.activation(
                                    out=dst, in_=ps, func=AF.Identity, bias=0.0, scale=1.0
                                )
                        else:
                            nc.vector.tensor_add(
                                out=dst,
                                in0=ps,
                                in1=vb_bc[:, oc * 512 : (oc + 1) * 512],
                            )

            if DBG:
                nc.sync.dma_start(out=dbg_qT[:], in_=qT)
                nc.sync.dma_start(out=dbg_kT[:], in_=kT)
                nc.sync.dma_start(out=dbg_v[:], in_=v)

            # ---- Stage B/C: attention per (head, q-chunk) ----
            with (
                tc.tile_pool(name="es_pool", bufs=2) as pes,
                tc.tile_pool(name="bc_pool", bufs=2) as pbc,
                tc.tile_pool(name="ps_sc", bufs=2, space="PSUM") as ps_sc,
                tc.tile_pool(name="ps_out", bufs=4, space="PSUM") as ps_out,
                tc.tile_pool(name="ps_den", bufs=2, space="PSUM") as ps_den,
            ):
                # The softmax denominator (ones-matmul, broadcast across all
                # 128 partitions by a full ones lhsT) + 1/den + out^T scaling
                # for unit u are emitted in the middle of unit u+1 (software
                # pipelining) so the latency never stalls the PE.

                def finalize_den(p):
                    es_p, ops_p, hqc = p
                    dps = ps_den.tile([P, 512], f32, tag="ps_den")
                    for jp in range(KT_TILES // 2):
                        nc.tensor.matmul(
                            dps,
                            lhsT=ones_k2,
                            rhs=es_p[:, 2 * jp : 2 * jp + 2, :],
                            start=(jp == 0),
                            stop=(jp == KT_TILES // 2 - 1),
                            perf_mode=DR,
                        )
                    bc = pbc.tile([P, 512], f32, tag="bc")
                    nc.vector.reciprocal(out=bc, in_=dps)
                    if DBG and hqc == (0, 0):
                        nc.sync.dma_start(out=dbg_bc[:], in_=bc)
                    return (ops_p, hqc, bc)

                def finalize_scale(p3):
                    ops_p, (h_p, qc_p), bc = p3
                    qsl_p = slice(qc_p * 512, (qc_p + 1) * 512)
                    for f in range(2):
                        nc.vector.tensor_mul(
                            out=outT[:, 2 * h_p + f, qsl_p], in0=ops_p[f], in1=bc
                        )

                pend = None
                pend2 = None
                for h in range(H):
                    for qc in range(NQ):
                        qsl = slice(qc * 512, (qc + 1) * 512)
                        es = pes.tile([P, KT_TILES, 512], fp8, tag="es")
                        op0 = ps_out.tile([P, 512], f32, tag="ps_out")
                        op1 = ps_out.tile([P, 512], f32, tag="ps_out")
                        ops = [op0, op1]
                        for j in range(KT_TILES):
                            sps = ps_sc.tile([P, 512], f32, tag="ps_sc")
                            nc.tensor.matmul(
                                sps,
                                lhsT=kT[:, 2 * h : 2 * h + 2, j * P : (j + 1) * P],
                                rhs=qT[:, 2 * h : 2 * h + 2, qsl],
                                start=True,
                                stop=True,
                                perf_mode=DR,
                            )
                            # bias -ln(32) scales es by 1/32 so the fp8 cast
                            # cannot overflow (hw fp8e4 infs above ~240); the
                            # softmax normalization cancels the factor exactly.
                            nc.scalar.activation(
                                out=es[:, j, :],
                                in_=sps,
                                func=AF.Exp,
                                bias=nln32,
                                scale=1.0 / 16.0,
                            )
                            if j == 4 and pend is not None:
                                pend2 = finalize_den(pend)
                                pend = None
                            if j == 11 and pend2 is not None:
                                finalize_scale(pend2)
                                pend2 = None
                            if j % 2 == 1:
                                jp = j // 2
                                for f in range(2):
                                    nc.tensor.matmul(
                                        ops[f],
                                        lhsT=v[
                                            :,
                                            2 * jp : 2 * jp + 2,
                                            h * HD + f * P : h * HD + (f + 1) * P,
                                        ],
                                        rhs=es[:, 2 * jp : 2 * jp + 2, :],
                                        start=(jp == 0),
                                        stop=(jp == KT_TILES // 2 - 1),
                                        perf_mode=DR,
                                    )
                        if DBG and h == 0 and qc == 0:
                            nc.sync.dma_start(out=dbg_es[:], in_=es)
                        pend = (es, ops, (h, qc))
                finalize_scale(finalize_den(pend))
                if DBG:
                    nc.sync.dma_start(out=dbg_outT[:], in_=outT)

            # ---- Stage D: wo projection + residual + LayerNorm ----
            with (
                tc.tile_pool(name="stD", bufs=3) as pd,
                tc.tile_pool(name="stD_y", bufs=2) as pdy,
                tc.tile_pool(name="stD_st", bufs=4) as pst,
                tc.tile_pool(name="ps_wo", bufs=4, space="PSUM") as ps_wo,
            ):
                nc.sync.dma_start(
                    out=woT, in_=woT_d[:].rearrange("(c p) o -> p c o", p=P)
                )
                if not ZB:
                    nc.sync.dma_start(out=gamma_bc, in_=gamma_d[:])
                    nc.sync.dma_start(out=beta_bc, in_=beta_d[:])
                for qt in range(QT_TILES):
                    y_t = pdy.tile([P, D], f32, tag="y")
                    for oc in range(2):
                        wps = ps_wo.tile([P, 512], f32, tag="ps_wo")
                        for dcp in range(DC // 2):
                            nc.tensor.matmul(
                                wps,
                                lhsT=outT[:, 2 * dcp : 2 * dcp + 2, qt * P : (qt + 1) * P],
                                rhs=woT[:, 2 * dcp : 2 * dcp + 2, oc * 512 : (oc + 1) * 512],
                                start=(dcp == 0),
                                stop=(dcp == DC // 2 - 1),
                                perf_mode=DR,
                            )
                        nc.scalar.activation(
                            out=y_t[:, oc * 512 : (oc + 1) * 512],
                            in_=wps,
                            func=AF.Identity,
                            bias=0.0,
                            scale=1.0,
                        )
                    # residual add in the DMA compute engine: y_t += xb
                    nc.gpsimd.dma_start(
                        out=y_t,
                        in_=xb_d[qt * P : (qt + 1) * P, :],
                        accum_op=ALU.add,
                    )
                    stats = pst.tile([P, 2, 6], f32, tag="stats")
                    for sg in range(2):
                        nc.vector.bn_stats(
                            out=stats[:, sg, :], in_=y_t[:, sg * 512 : (sg + 1) * 512]
                        )
                    mv = pst.tile([P, 2], f32, tag="mv")
                    nc.vector.bn_aggr(out=mv, in_=stats)
                    std = pst.tile([P, 1], f32, tag="std")
                    nc.scalar.activation(
                        out=std, in_=mv[:, 1:2], func=AF.Sqrt, bias=eps_t, scale=1.0
                    )
                    rstd = pst.tile([P, 1], f32, tag="rstd")
                    nc.vector.reciprocal(out=rstd, in_=std)
                    nc.vector.tensor_scalar(
                        out=y_t,
                        in0=y_t,
                        scalar1=mv[:, 0:1],
                        scalar2=rstd,
                        op0=ALU.subtract,
                        op1=ALU.mult,
                    )
                    if not ZB:
                        nc.vector.tensor_mul(out=y_t, in0=y_t, in1=gamma_bc)
                        nc.vector.tensor_add(out=y_t, in0=y_t, in1=beta_bc)
                    nc.sync.dma_start(out=y_d[qt * P : (qt + 1) * P, :], in_=y_t)

    _split_excess_waits(nc)
    return nc


def _get_nc(ZB):
    key = ("nc", ZB)
    if key not in _CACHE:
        _CACHE[key] = _build(ZB)
    return _CACHE[key]


def kernel(x, qkv_w, qkv_b, wo_w, wo_b, gamma, beta):
    from concourse.bass_utils import run_bass_kernel_spmd

    x = np.asarray(x, dtype=np.float32)
    qkv_w = np.asarray(qkv_w, dtype=np.float32)
    qkv_b = np.asarray(qkv_b, dtype=np.float32)
    wo_w = np.asarray(wo_w, dtype=np.float32)
    wo_b = np.asarray(wo_b, dtype=np.float32)
    gamma = np.asarray(gamma, dtype=np.float32)
    beta = np.asarray(beta, dtype=np.float32)

    wqkvT = np.ascontiguousarray(qkv_w.T).astype(_FP8)
    woT = np.ascontiguousarray(wo_w.T).astype(_FP8)
    qkvb2d = np.ascontiguousarray(qkv_b.reshape(24, P).T)
    vb = np.ascontiguousarray(np.broadcast_to(qkv_b[2 * D :], (P, D))).astype(np.float32)
    gamma_r = np.ascontiguousarray(np.broadcast_to(gamma, (P, D))).astype(np.float32)
    beta_r = np.ascontiguousarray(np.broadcast_to(beta, (P, D))).astype(np.float32)

    in_maps = []
    for c in range(NCORES):
        b, t = divmod(c, 2)
        loc = x[b, t * SL : (t + 1) * SL]  # [SL, D]
        oth = x[b, (1 - t) * SL : (2 - t) * SL]
        xT = np.concatenate([loc.T, oth.T], axis=1).astype(_FP8)  # [D, S]
        xb = (loc + wo_b[None, :]).astype(np.float32)
        in_maps.append(
            {
                "xT": np.ascontiguousarray(xT),
                "wqkvT": wqkvT,
                "woT": woT,
                "xb": xb,
                "qkvb": qkvb2d,
                "vb": vb,
                "gamma": gamma_r,
                "beta": beta_r,
            }
        )

    zb = (
        not qkv_b.any()
        and bool(np.all(gamma == 1.0))
        and not beta.any()
    )
    trace = os.environ.get("BASS_KERNEL_TRACE") == "1"
    res = run_bass_kernel_spmd(
        _get_nc(zb),
        in_maps,
        list(range(NCORES)),
        trace=trace,
        tmpdir=os.environ.get("BASS_KERNEL_TRACE_DIR") or None,
    )
    _CACHE["last_result"] = res

    out = np.empty((B, S, D), dtype=np.float32)
    for c in range(NCORES):
        b, t = divmod(c, 2)
        out[b, t * SL : (t + 1) * SL] = res.results[c]["y"]
    return out


# revision 17
# speedup vs baseline: 1.5936x; 1.1088x over previous
"""Trainium2 Bass kernel: 4-head transformer core (attention + residual + LayerNorm).

Reference computation (per batch b of 4, seq 2048, d_model 1024, 4 heads x 256):
    qkv = x @ qkv_w.T + qkv_b ; q,k,v per head
    attn = softmax(q k^T / 16) ; out = attn v
    y = x + out @ wo_w.T + wo_b ; layernorm(y) * gamma + beta

Sharding: pure data parallel over (batch, seq-half) -> 8 cores, no collectives.
Each core handles 1024 query tokens of one batch; K/V are computed for the
full 2048 tokens of that batch (duplicated across the 2 cores of a batch).
Host passes x pre-transposed (d-major) and rotated so the core's local tokens
are always columns [0, 1024) -- one SPMD program serves all cores.  Attention
is permutation-invariant over key/value positions, so the rotation does not
change the result.

All matmuls run in fp8e4 (e4m3) with DoubleRow perf mode: each instruction
contracts K=256 (two 128-partition subtiles packed in the free dim of both
operands) at the same per-instruction cost as one bf16 K=128 matmul -- 2x
the effective PE throughput.  The attention output is diluted ~25x by the
f32 residual before LayerNorm, so fp8 quantization of x/q/k/v/es/wo keeps
the final relative error ~4e-3, well inside the 2e-2 gate.  exp carries a
-ln(32) bias so its fp8 output cannot overflow (hw fp8e4 infs above ~240);
the softmax normalization cancels the factor exactly.

Schedule: ONE fully interleaved instruction stream.  Attention units run
qc-outer / h-inner; the QKV projection chunks for head h+1 (and the V
chunks) are paced INSIDE unit h's j-loop so the PE never waits on the ACT
exp chain, and the wo projection + LayerNorm for the qc0 token tiles are
paced inside the qc1 units.  Only the last 4 token tiles drain after the
final attention unit.  PSUM: scores 2 banks, av accumulators 4, misc
(QKV/V/wo chunks + softmax denominator) 2.

Softmax denominator: DoubleRow ones-matmul broadcasts den to all 128
partitions; 1/den via the fast DVE Newton reciprocal; out^T = av * (1/den)
on DVE (fp8 out).  LayerNorm rstd = exp(-0.5*ln(var+eps)) on ACT -- Ln and
Exp live in the same ACT table set, so no table reloads mid-kernel.
"""

import os

import ml_dtypes
import numpy as np

P = 128
B, S, D = 4, 2048, 1024
H = 4
HD = D // H  # 256
SL = S // 2  # local query tokens per core
DC = D // P  # 8 d-chunks
QT_TILES = SL // P  # 8
KT_TILES = S // P  # 16
NQ = SL // 512  # 2 q-chunks of 512
EPS = 1e-5
NCORES = 8

_BF16 = ml_dtypes.bfloat16
_FP8 = ml_dtypes.float8_e4m3fn

_CACHE = {}


def _install_drain_patch():
    """walrus CoreV3 in this container accepts at most one sem wait per SP
    CTRL instruction, but Tile's kernel-tail drain carries one wait per
    outstanding logical proc.  Redistribute them onto single-wait no-ops."""
    import concourse.tile as _tile
    from concourse import mybir
    from concourse.vector_clock import ScopedClock

    if getattr(_tile.TileContext, "_drain_patch_installed", False):
        return

    def _drain_and_barrier(self, tick_clock, wait_clock):
        nc = self.nc
        drain_inst = nc.sync.drain()
        wait_clock.add_sem_waits(
            drain_inst.ins, ScopedClock({None: tick_clock.global_clock})
        )
        si = drain_inst.ins.sync_info
        if si is not None and len(si.on_wait) > 1:
            waits = list(si.on_wait)
            drain_inst.ins.sync_info = mybir.SyncInfo(
                on_wait=[], on_update=list(si.on_update)
            )
            for w in waits:
                nop = nc.sync.nop(nofuse=True, hint="drain_wait_split")
                nop.ins.sync_info = mybir.SyncInfo(on_wait=[w], on_update=[])

        nc.all_engine_barrier()
        assert self.sems is not None
        popped = nc._tile_sem_poison_stack.pop()
        assert popped is self._sem_poison
        nc.clear_and_free_semaphores(list(self.sems.allocated().values()))
        nc.all_engine_barrier()

    _tile.TileContext._drain_and_barrier = _drain_and_barrier
    _tile.TileContext._drain_patch_installed = True


def _split_excess_waits(nc):
    """This walrus build accepts at most one sem wait per instruction (two for
    EventSemaphore), but Tile attaches one wait per outstanding proc.  Move
    the excess waits onto same-engine no-ops inserted just before each
    over-subscribed instruction (same-engine program order makes the waits
    complete before the instruction issues)."""
    from concourse import mybir

    n_split = 0
    for f in nc.m.functions:
        for b in f.blocks:
            insts = b.instructions
            new_list = []
            changed = False
            for inst in insts:
                si = inst.sync_info
                cap = 2 if isinstance(inst, mybir.InstEventSemaphore) else 1
                if si is not None and len(si.on_wait) > cap:
                    waits = list(si.on_wait)
                    for k, w in enumerate(waits[:-cap]):
                        nop = mybir.InstNoOp(name=f"{inst.name}-ws{k}")
                        nop.engine = inst.engine
                        nop.bass_nofuse = True
                        nop.sync_info = mybir.SyncInfo(on_wait=[w], on_update=[])
                        new_list.append(nop)
                        n_split += 1
                    inst.sync_info = mybir.SyncInfo(
                        on_wait=waits[-cap:], on_update=list(si.on_update)
                    )
                    changed = True
                new_list.append(inst)
            if changed:
                b.instructions = new_list
    return n_split


def _build(ZB=False):
    """ZB: specialize for qkv_b == 0, wo_b folded on host, gamma == 1, beta == 0."""
    import concourse.bass as bass
    import concourse.tile as tile
    from concourse import mybir

    _install_drain_patch()

    f32 = mybir.dt.float32
    fp8 = mybir.dt.float8e4
    AF = mybir.ActivationFunctionType
    ALU = mybir.AluOpType
    DR = mybir.MatmulPerfMode.DoubleRow

    nc = bass.Bass()

    xT_d = nc.dram_tensor("xT", [D, S], fp8, kind="ExternalInput")
    wqkvT_d = nc.dram_tensor("wqkvT", [D, 3 * D], fp8, kind="ExternalInput")
    woT_d = nc.dram_tensor("woT", [D, D], fp8, kind="ExternalInput")
    xb_d = nc.dram_tensor("xb", [SL, D], f32, kind="ExternalInput")
    qkvb_d = nc.dram_tensor("qkvb", [P, 24], f32, kind="ExternalInput")
    vb_d = nc.dram_tensor("vb", [P, D], f32, kind="ExternalInput")
    gamma_d = nc.dram_tensor("gamma", [P, D], f32, kind="ExternalInput")
    beta_d = nc.dram_tensor("beta", [P, D], f32, kind="ExternalInput")
    y_d = nc.dram_tensor("y", [SL, D], f32, kind="ExternalOutput")

    with (
        tile.TileContext(nc) as tc,
        nc.allow_low_precision(reason="fp8 attention path, tolerance 2e-2"),
        tc.tile_pool(name="persist", bufs=1) as pp,
        tc.tile_pool(name="es_pool", bufs=2) as pes,
        tc.tile_pool(name="bc_pool", bufs=2) as pbc,
        tc.tile_pool(name="y_pool", bufs=3) as pdy,
        tc.tile_pool(name="st_pool", bufs=4) as pst,
        tc.tile_pool(name="ps_sc", bufs=2, space="PSUM") as ps_sc,
        tc.tile_pool(name="ps_out", bufs=4, space="PSUM") as ps_out,
        tc.tile_pool(name="ps_misc", bufs=2, space="PSUM") as ps_misc,
    ):
        qT = pp.tile([P, DC, SL], fp8, tag="qT")
        kT = pp.tile([P, DC, S], fp8, tag="kT")
        v = pp.tile([P, KT_TILES, D], fp8, tag="v")
        outT = pp.tile([P, DC, SL], fp8, tag="outT")
        woT = pp.tile([P, DC, D], fp8, tag="woT")
        xT = pp.tile([P, DC, S], fp8, tag="xT")
        wv = pp.tile([P, DC, D], fp8, tag="wv")
        wqk = [pp.tile([P, DC, P], fp8, name=f"wqk{m}", tag=f"wqk{m}") for m in range(16)]
        if not ZB:
            gamma_bc = pp.tile([P, D], f32, tag="gamma_bc")
            beta_bc = pp.tile([P, D], f32, tag="beta_bc")
            vb_bc = pp.tile([P, D], f32, tag="vb_bc")
            qkvb = pp.tile([P, 24], f32, tag="qkvb")
        ones_k2 = pp.tile([P, 2, P], fp8, tag="ones_k2")
        eps_t = pp.tile([P, 1], f32, tag="eps")
        nln32 = pp.tile([P, 1], f32, tag="nln32")

        nc.vector.memset(ones_k2, 1.0)
        nc.vector.memset(eps_t, EPS)
        nc.vector.memset(nln32, -3.4657359027997265)

        # ---- input DMAs, first-use order ----
        wqkvT_r = wqkvT_d[:].rearrange("(c p) f -> p c f", p=P)
        xT_r = xT_d[:].rearrange("(c p) t -> p c t", p=P)
        nc.scalar.dma_start(out=xT[:, :, 0:512], in_=xT_r[:, :, 0:512])
        nc.sync.dma_start(out=wqk[8], in_=wqkvT_r[:, :, 8 * P : 9 * P])
        nc.sync.dma_start(out=wqk[9], in_=wqkvT_r[:, :, 9 * P : 10 * P])
        nc.sync.dma_start(out=wqk[0], in_=wqkvT_r[:, :, 0:P])
        nc.sync.dma_start(out=wqk[1], in_=wqkvT_r[:, :, P : 2 * P])
        for tch in range(1, 4):
            nc.scalar.dma_start(
                out=xT[:, :, tch * 512 : (tch + 1) * 512],
                in_=xT_r[:, :, tch * 512 : (tch + 1) * 512],
            )
        nc.gpsimd.dma_start(out=wv, in_=wqkvT_r[:, :, 2 * D : 3 * D])
        for m in (10, 11, 2, 3, 12, 13, 4, 5, 14, 15, 6, 7):
            nc.sync.dma_start(out=wqk[m], in_=wqkvT_r[:, :, m * P : (m + 1) * P])
        nc.gpsimd.dma_start(out=woT, in_=woT_d[:].rearrange("(c p) o -> p c o", p=P))
        if not ZB:
            nc.gpsimd.dma_start(out=qkvb, in_=qkvb_d[:])
            nc.gpsimd.dma_start(out=vb_bc, in_=vb_d[:])
            nc.sync.dma_start(out=gamma_bc, in_=gamma_d[:])
            nc.sync.dma_start(out=beta_bc, in_=beta_d[:])

        # ---- chunk emitters (each: 4 DoubleRow matmuls + one PSUM->SBUF cast) ----
        def qkv_chunk(m, qc, eng):
            """Q (m<8, 512 local tokens) or K (m>=8, 512 of 2048 tokens)."""
            ps = ps_misc.tile([P, 512], f32, tag="ps_misc")
            for dcp in range(DC // 2):
                nc.tensor.matmul(
                    ps,
                    lhsT=wqk[m][:, 2 * dcp : 2 * dcp + 2, :],
                    rhs=xT[:, 2 * dcp : 2 * dcp + 2, qc * 512 : (qc + 1) * 512],
                    start=(dcp == 0),
                    stop=(dcp == DC // 2 - 1),
                    perf_mode=DR,
                )
            if m < 8:
                dst = qT[:, m, qc * 512 : (qc + 1) * 512]
            else:
                dst = kT[:, m - 8, qc * 512 : (qc + 1) * 512]
            if ZB:
                if eng is nc.scalar:
                    nc.scalar.activation(
                        out=dst, in_=ps, func=AF.Identity, bias=0.0, scale=1.0
                    )
                else:
                    eng.tensor_copy(out=dst, in_=ps)
            else:
                nc.scalar.activation(
                    out=dst, in_=ps, func=AF.Identity, bias=qkvb[:, m : m + 1], scale=1.0
                )

        def v_chunk(vt, oc, eng):
            ps = ps_misc.tile([P, 512], f32, tag="ps_misc")
            for dcp in range(DC // 2):
                nc.tensor.matmul(
                    ps,
                    lhsT=xT[:, 2 * dcp : 2 * dcp + 2, vt * P : (vt + 1) * P],
                    rhs=wv[:, 2 * dcp : 2 * dcp + 2, oc * 512 : (oc + 1) * 512],
                    start=(dcp == 0),
                    stop=(dcp == DC // 2 - 1),
                    perf_mode=DR,
                )
            dst = v[:, vt, oc * 512 : (oc + 1) * 512]
            if ZB:
                if eng is nc.scalar:
                    nc.scalar.activation(
                        out=dst, in_=ps, func=AF.Identity, bias=0.0, scale=1.0
                    )
                else:
                    eng.tensor_copy(out=dst, in_=ps)
            else:
                nc.vector.tensor_add(
                    out=dst, in0=ps, in1=vb_bc[:, oc * 512 : (oc + 1) * 512]
                )

        y_tiles = {}

        def wo_chunk(qt, oc, eng):
            if qt not in y_tiles:
                y_tiles[qt] = pdy.tile([P, D], f32, name=f"y{qt}", tag="y")
            y_t = y_tiles[qt]
            ps = ps_misc.tile([P, 512], f32, tag="ps_misc")
            for dcp in range(DC // 2):
                nc.tensor.matmul(
                    ps,
                    lhsT=outT[:, 2 * dcp : 2 * dcp + 2, qt * P : (qt + 1) * P],
                    rhs=woT[:, 2 * dcp : 2 * dcp + 2, oc * 512 : (oc + 1) * 512],
                    start=(dcp == 0),
                    stop=(dcp == DC // 2 - 1),
                    perf_mode=DR,
                )
            dst = y_t[:, oc * 512 : (oc + 1) * 512]
            if eng is nc.scalar:
                nc.scalar.activation(out=dst, in_=ps, func=AF.Identity, bias=0.0, scale=1.0)
            else:
                eng.tensor_copy(out=dst, in_=ps)

        def ln_tile(qt):
            """residual add + LayerNorm + store for token tile qt."""
            y_t = y_tiles.pop(qt)
            nc.gpsimd.dma_start(
                out=y_t, in_=xb_d[qt * P : (qt + 1) * P, :], accum_op=ALU.add
            )
            stats = pst.tile([P, 2, 6], f32, tag="stats")
            for sg in range(2):
                nc.vector.bn_stats(
                    out=stats[:, sg, :], in_=y_t[:, sg * 512 : (sg + 1) * 512]
                )
            mv = pst.tile([P, 2], f32, tag="mv")
            nc.vector.bn_aggr(out=mv, in_=stats)
            # rstd = exp(-0.5*ln(var+eps)): Ln and Exp share the ACT exp
            # table set, so this never forces a mid-kernel table reload.
            lnv = pst.tile([P, 1], f32, tag="lnv")
            nc.scalar.activation(
                out=lnv, in_=mv[:, 1:2], func=AF.Ln, bias=eps_t, scale=1.0
            )
            rstd = pst.tile([P, 1], f32, tag="rstd")
            nc.scalar.activation(out=rstd, in_=lnv, func=AF.Exp, bias=0.0, scale=-0.5)
            nc.vector.tensor_scalar(
                out=y_t,
                in0=y_t,
                scalar1=mv[:, 0:1],
                scalar2=rstd,
                op0=ALU.subtract,
                op1=ALU.mult,
            )
            if not ZB:
                nc.vector.tensor_mul(out=y_t, in0=y_t, in1=gamma_bc)
                nc.vector.tensor_add(out=y_t, in0=y_t, in1=beta_bc)
            nc.sync.dma_start(out=y_d[qt * P : (qt + 1) * P, :], in_=y_t)

        # ---- softmax finalize (pipelined into the following unit) ----
        def finalize_den(p):
            es_p, ops_p, hqc = p
            dps = ps_misc.tile([P, 512], f32, name="dps", tag="ps_misc")
            for jp in range(KT_TILES // 2):
                nc.tensor.matmul(
                    dps,
                    lhsT=ones_k2,
                    rhs=es_p[:, 2 * jp : 2 * jp + 2, :],
                    start=(jp == 0),
                    stop=(jp == KT_TILES // 2 - 1),
                    perf_mode=DR,
                )
            bc = pbc.tile([P, 512], f32, tag="bc")
            nc.vector.reciprocal(out=bc, in_=dps)
            return (ops_p, hqc, bc)

        def finalize_scale(p2):
            ops_p, (qc_p, h_p), bc = p2
            qsl_p = slice(qc_p * 512, (qc_p + 1) * 512)
            for f in range(2):
                nc.vector.tensor_mul(
                    out=outT[:, 2 * h_p + f, qsl_p], in0=ops_p[f], in1=bc
                )

        # ---- the interleaved main loop ----
        state = {"pend": None, "pend2": None}

        def unit(qc, h, carried, start_at=0):
            """One attention unit (512 q tokens x head h), with `carried`
            thunks (projection/wo/LN work) paced across iterations
            [start_at, 16) of its j-loop."""
            n_car = len(carried)
            emitted = 0
            qsl = slice(qc * 512, (qc + 1) * 512)
            es = pes.tile([P, KT_TILES, 512], fp8, tag="es")
            op0 = ps_out.tile([P, 512], f32, tag="ps_out")
            op1 = ps_out.tile([P, 512], f32, tag="ps_out")
            ops = [op0, op1]
            for j in range(KT_TILES):
                sps = ps_sc.tile([P, 512], f32, tag="ps_sc")
                nc.tensor.matmul(
                    sps,
                    lhsT=kT[:, 2 * h : 2 * h + 2, j * P : (j + 1) * P],
                    rhs=qT[:, 2 * h : 2 * h + 2, qsl],
                    start=True,
                    stop=True,
                    perf_mode=DR,
                )
                nc.scalar.activation(
                    out=es[:, j, :],
                    in_=sps,
                    func=AF.Exp,
                    bias=nln32,
                    scale=1.0 / 16.0,
                )
                if j == 4 and state["pend"] is not None:
                    state["pend2"] = finalize_den(state["pend"])
                    state["pend"] = None
                if j == 11 and state["pend2"] is not None:
                    finalize_scale(state["pend2"])
                    state["pend2"] = None
                want = n_car * max(0, j + 1 - start_at) // (KT_TILES - start_at)
                while emitted < want:
                    carried[emitted]()
                    emitted += 1
                if j % 2 == 1:
                    jp = j // 2
                    for f in range(2):
                        nc.tensor.matmul(
                            ops[f],
                            lhsT=v[
                                :,
                                2 * jp : 2 * jp + 2,
                                h * HD + f * P : h * HD + (f + 1) * P,
                            ],
                            rhs=es[:, 2 * jp : 2 * jp + 2, :],
                            start=(jp == 0),
                            stop=(jp == KT_TILES // 2 - 1),
                            perf_mode=DR,
                        )
            while emitted < n_car:
                carried[emitted]()
                emitted += 1
            state["pend"] = (es, ops, (qc, h))

        A, V_ = nc.scalar, nc.vector

        # PRE: everything unit (qc0,h0) needs up front.
        pre = [
            lambda: qkv_chunk(8, 0, A),
            lambda: qkv_chunk(9, 0, V_),
            lambda: qkv_chunk(0, 0, A),
            lambda: qkv_chunk(1, 0, V_),
            lambda: qkv_chunk(8, 1, A),
            lambda: qkv_chunk(9, 1, V_),
            lambda: qkv_chunk(8, 2, A),
            lambda: qkv_chunk(9, 2, V_),
            lambda: qkv_chunk(8, 3, A),
            lambda: qkv_chunk(9, 3, V_),
            lambda: v_chunk(0, 0, A),
            lambda: v_chunk(1, 0, V_),
            lambda: v_chunk(2, 0, A),
            lambda: v_chunk(3, 0, V_),
        ]
        for t in pre:
            t()

        def carry_k(h2):  # K chunks for head h2 (m = 8+2*h2, 9+2*h2), token order
            out = []
            for qc2 in range(4):
                out.append(lambda m=8 + 2 * h2, q2=qc2: qkv_chunk(m, q2, V_))
                out.append(lambda m=9 + 2 * h2, q2=qc2: qkv_chunk(m, q2, V_))
            return out

        def carry_q(h2, qc2):
            return [
                lambda m=2 * h2: qkv_chunk(m, qc2, V_),
                lambda m=2 * h2 + 1: qkv_chunk(m, qc2, V_),
            ]

        def carry_v(vts, oc):
            return [lambda t=t_, o=oc: v_chunk(t, o, V_) for t_ in vts]

        carried_by_unit = [
            # (qc0,h0): rest of v oc0 (self, paced ahead of av) + head1 k/q
            carry_v(range(4, 16), 0) + carry_k(1) + carry_q(1, 0),
            # (qc0,h1): v oc1 first half + head2 k/q
            carry_v(range(0, 8), 1) + carry_k(2) + carry_q(2, 0),
            # (qc0,h2): v oc1 second half (self-paced, needed from jp>=4) + head3 k/q
            carry_v(range(8, 16), 1) + carry_k(3) + carry_q(3, 0),
            # (qc0,h3): all qc1 q chunks
            [t for h2 in range(4) for t in carry_q(h2, 1)],
            # (qc1,h0): nothing until the qc0 finalize lands (j==11); wo qt0
            [lambda: wo_chunk(0, 0, V_), lambda: wo_chunk(0, 1, V_), lambda: ln_tile(0)],
            # (qc1,h1): wo qt1
            [lambda: wo_chunk(1, 0, V_), lambda: wo_chunk(1, 1, V_), lambda: ln_tile(1)],
            # (qc1,h2): wo qt2
            [lambda: wo_chunk(2, 0, V_), lambda: wo_chunk(2, 1, V_), lambda: ln_tile(2)],
            # (qc1,h3): wo qt3
            [lambda: wo_chunk(3, 0, V_), lambda: wo_chunk(3, 1, V_), lambda: ln_tile(3)],
        ]
        units = [(qc, h) for qc in range(NQ) for h in range(H)]
        for (qc, h), carried in zip(units, carried_by_unit, strict=True):
            # (qc1,h0)'s wo work is legal only after the (qc0,h3)
            # finalize_scale, which this unit emits at j==11.
            unit(qc, h, carried, start_at=12 if (qc, h) == (1, 0) else 0)

        # drain: finalize last unit, then the remaining 4 token tiles.
        finalize_scale(finalize_den(state["pend"]))
        for qt in range(4, 8):
            wo_chunk(qt, 0, A)
            wo_chunk(qt, 1, V_)
            ln_tile(qt)

    _split_excess_waits(nc)
    return nc


def _get_nc(ZB):
    key = ("nc", ZB)
    if key not in _CACHE:
        _CACHE[key] = _build(ZB)
    return _CACHE[key]


def kernel(x, qkv_w, qkv_b, wo_w, wo_b, gamma, beta):
    from concourse.bass_utils import run_bass_kernel_spmd

    x = np.asarray(x, dtype=np.float32)
    qkv_w = np.asarray(qkv_w, dtype=np.float32)
    qkv_b = np.asarray(qkv_b, dtype=np.float32)
    wo_w = np.asarray(wo_w, dtype=np.float32)
    wo_b = np.asarray(wo_b, dtype=np.float32)
    gamma = np.asarray(gamma, dtype=np.float32)
    beta = np.asarray(beta, dtype=np.float32)

    wqkvT = np.ascontiguousarray(qkv_w.T).astype(_FP8)
    woT = np.ascontiguousarray(wo_w.T).astype(_FP8)
    qkvb2d = np.ascontiguousarray(qkv_b.reshape(24, P).T)
    vb = np.ascontiguousarray(np.broadcast_to(qkv_b[2 * D :], (P, D))).astype(np.float32)
    gamma_r = np.ascontiguousarray(np.broadcast_to(gamma, (P, D))).astype(np.float32)
    beta_r = np.ascontiguousarray(np.broadcast_to(beta, (P, D))).astype(np.float32)

    in_maps = []
    for c in range(NCORES):
        b, t = divmod(c, 2)
        loc = x[b, t * SL : (t + 1) * SL]  # [SL, D]
        oth = x[b, (1 - t) * SL : (2 - t) * SL]
        xT = np.concatenate([loc.T, oth.T], axis=1).astype(_FP8)  # [D, S]
        xb = (loc + wo_b[None, :]).astype(np.float32)
        in_maps.append(
            {
                "xT": np.ascontiguousarray(xT),
                "wqkvT": wqkvT,
                "woT": woT,
                "xb": xb,
                "qkvb": qkvb2d,
                "vb": vb,
                "gamma": gamma_r,
                "beta": beta_r,
            }
        )

    zb = (
        not qkv_b.any()
        and bool(np.all(gamma == 1.0))
        and not beta.any()
    )
    trace = os.environ.get("BASS_KERNEL_TRACE") == "1"
    res = run_bass_kernel_spmd(
        _get_nc(zb),
        in_maps,
        list(range(NCORES)),
        trace=trace,
        tmpdir=os.environ.get("BASS_KERNEL_TRACE_DIR") or None,
    )
    _CACHE["last_result"] = res

    out = np.empty((B, S, D), dtype=np.float32)
    for c in range(NCORES):
        b, t = divmod(c, 2)
        out[b, t * SL : (t + 1) * SL] = res.results[c]["y"]
    return out


# revision 18
# speedup vs baseline: 1.7446x; 1.0947x over previous
"""Trainium2 Bass kernel: 4-head transformer core (attention + residual + LayerNorm).

Reference computation (per batch b of 4, seq 2048, d_model 1024, 4 heads x 256):
    qkv = x @ qkv_w.T + qkv_b ; q,k,v per head
    attn = softmax(q k^T / 16) ; out = attn v
    y = x + out @ wo_w.T + wo_b ; layernorm(y) * gamma + beta

Sharding: pure data parallel over (batch, seq-half) -> 8 cores, no collectives.
Each core handles 1024 query tokens of one batch; K/V are computed for the
full 2048 tokens of that batch (duplicated across the 2 cores of a batch).
Host passes x pre-transposed (d-major) and rotated so the core's local tokens
are always columns [0, 1024) -- one SPMD program serves all cores.  Attention
is permutation-invariant over key/value positions, so the rotation does not
change the result.

All matmuls run in fp8e4 (e4m3) with DoubleRow perf mode: each instruction
contracts K=256 (two 128-partition subtiles packed in the free dim of both
operands) at the same per-instruction cost as one bf16 K=128 matmul -- 2x
the effective PE throughput.  The attention output is diluted ~25x by the
f32 residual before LayerNorm, so fp8 quantization of x/q/k/v/es/wo keeps
the final relative error ~4e-3, well inside the 2e-2 gate.  exp carries a
-ln(32) bias so its fp8 output cannot overflow (hw fp8e4 infs above ~240);
the softmax normalization cancels the factor exactly.

Schedule: ONE fully interleaved instruction stream.  Attention units run
qc-outer / h-inner; the QKV projection chunks for head h+1 (and the V
chunks) are paced INSIDE unit h's j-loop so the PE never waits on the ACT
exp chain, and the wo projection + LayerNorm for the qc0 token tiles are
paced inside the qc1 units.  Only the last 4 token tiles drain after the
final attention unit.  PSUM: scores 2 banks, av accumulators 4, misc
(QKV/V/wo chunks + softmax denominator) 2.

Softmax denominator: DoubleRow ones-matmul broadcasts den to all 128
partitions; 1/den via the fast DVE Newton reciprocal; out^T = av * (1/den)
on DVE (fp8 out).  LayerNorm rstd = exp(-0.5*ln(var+eps)) on ACT -- Ln and
Exp live in the same ACT table set, so no table reloads mid-kernel.
"""

import os

import ml_dtypes
import numpy as np

P = 128
B, S, D = 4, 2048, 1024
H = 4
HD = D // H  # 256
SL = S // 2  # local query tokens per core
DC = D // P  # 8 d-chunks
QT_TILES = SL // P  # 8
KT_TILES = S // P  # 16
NQ = SL // 512  # 2 q-chunks of 512
EPS = 1e-5
NCORES = 8

_BF16 = ml_dtypes.bfloat16
_FP8 = ml_dtypes.float8_e4m3fn

_CACHE = {}


def _install_drain_patch():
    """walrus CoreV3 in this container accepts at most one sem wait per SP
    CTRL instruction, but Tile's kernel-tail drain carries one wait per
    outstanding logical proc.  Redistribute them onto single-wait no-ops."""
    import concourse.tile as _tile
    from concourse import mybir
    from concourse.vector_clock import ScopedClock

    if getattr(_tile.TileContext, "_drain_patch_installed", False):
        return

    def _drain_and_barrier(self, tick_clock, wait_clock):
        nc = self.nc
        drain_inst = nc.sync.drain()
        wait_clock.add_sem_waits(
            drain_inst.ins, ScopedClock({None: tick_clock.global_clock})
        )
        si = drain_inst.ins.sync_info
        if si is not None and len(si.on_wait) > 1:
            waits = list(si.on_wait)
            drain_inst.ins.sync_info = mybir.SyncInfo(
                on_wait=[], on_update=list(si.on_update)
            )
            for w in waits:
                nop = nc.sync.nop(nofuse=True, hint="drain_wait_split")
                nop.ins.sync_info = mybir.SyncInfo(on_wait=[w], on_update=[])

        nc.all_engine_barrier()
        assert self.sems is not None
        popped = nc._tile_sem_poison_stack.pop()
        assert popped is self._sem_poison
        nc.clear_and_free_semaphores(list(self.sems.allocated().values()))
        nc.all_engine_barrier()

    _tile.TileContext._drain_and_barrier = _drain_and_barrier
    _tile.TileContext._drain_patch_installed = True


def _split_excess_waits(nc):
    """This walrus build accepts at most one sem wait per instruction (two for
    EventSemaphore), but Tile attaches one wait per outstanding proc.  Move
    the excess waits onto same-engine no-ops inserted just before each
    over-subscribed instruction (same-engine program order makes the waits
    complete before the instruction issues)."""
    from concourse import mybir

    n_split = 0
    for f in nc.m.functions:
        for b in f.blocks:
            insts = b.instructions
            new_list = []
            changed = False
            for inst in insts:
                si = inst.sync_info
                cap = 2 if isinstance(inst, mybir.InstEventSemaphore) else 1
                if si is not None and len(si.on_wait) > cap:
                    waits = list(si.on_wait)
                    for k, w in enumerate(waits[:-cap]):
                        nop = mybir.InstNoOp(name=f"{inst.name}-ws{k}")
                        nop.engine = inst.engine
                        nop.bass_nofuse = True
                        nop.sync_info = mybir.SyncInfo(on_wait=[w], on_update=[])
                        new_list.append(nop)
                        n_split += 1
                    inst.sync_info = mybir.SyncInfo(
                        on_wait=waits[-cap:], on_update=list(si.on_update)
                    )
                    changed = True
                new_list.append(inst)
            if changed:
                b.instructions = new_list
    return n_split


def _build(ZB=False):
    """ZB: specialize for qkv_b == 0, wo_b folded on host, gamma == 1, beta == 0."""
    import concourse.bass as bass
    import concourse.tile as tile
    from concourse import mybir

    _install_drain_patch()

    f32 = mybir.dt.float32
    fp8 = mybir.dt.float8e4
    AF = mybir.ActivationFunctionType
    ALU = mybir.AluOpType
    DR = mybir.MatmulPerfMode.DoubleRow

    nc = bass.Bass()

    xT_d = nc.dram_tensor("xT", [D, S], fp8, kind="ExternalInput")
    wqkvT_d = nc.dram_tensor("wqkvT", [D, 3 * D], fp8, kind="ExternalInput")
    woT_d = nc.dram_tensor("woT", [D, D], fp8, kind="ExternalInput")
    xb_d = nc.dram_tensor("xb", [SL, D], f32, kind="ExternalInput")
    qkvb_d = nc.dram_tensor("qkvb", [P, 24], f32, kind="ExternalInput")
    vb_d = nc.dram_tensor("vb", [P, D], f32, kind="ExternalInput")
    gamma_d = nc.dram_tensor("gamma", [P, D], f32, kind="ExternalInput")
    beta_d = nc.dram_tensor("beta", [P, D], f32, kind="ExternalInput")
    y_d = nc.dram_tensor("y", [SL, D], f32, kind="ExternalOutput")

    with (
        tile.TileContext(nc) as tc,
        nc.allow_low_precision(reason="fp8 attention path, tolerance 2e-2"),
        tc.tile_pool(name="persist", bufs=1) as pp,
        tc.tile_pool(name="es_pool", bufs=2) as pes,
        tc.tile_pool(name="bc_pool", bufs=2) as pbc,
        tc.tile_pool(name="y_pool", bufs=3) as pdy,
        tc.tile_pool(name="st_pool", bufs=4) as pst,
        tc.tile_pool(name="ps_sc", bufs=2, space="PSUM") as ps_sc,
        tc.tile_pool(name="ps_out", bufs=4, space="PSUM") as ps_out,
        tc.tile_pool(name="ps_misc", bufs=2, space="PSUM") as ps_misc,
    ):
        qT = pp.tile([P, DC, SL], fp8, tag="qT")
        kT = pp.tile([P, DC, S], fp8, tag="kT")
        v = pp.tile([P, KT_TILES, D], fp8, tag="v")
        outT = pp.tile([P, DC, SL], fp8, tag="outT")
        woT = pp.tile([P, DC, D], fp8, tag="woT")
        xT = pp.tile([P, DC, S], fp8, tag="xT")
        wv = pp.tile([P, DC, D], fp8, tag="wv")
        wqk = [pp.tile([P, DC, P], fp8, name=f"wqk{m}", tag=f"wqk{m}") for m in range(16)]
        if not ZB:
            gamma_bc = pp.tile([P, D], f32, tag="gamma_bc")
            beta_bc = pp.tile([P, D], f32, tag="beta_bc")
            vb_bc = pp.tile([P, D], f32, tag="vb_bc")
            qkvb = pp.tile([P, 24], f32, tag="qkvb")
        ones_k2 = pp.tile([P, 2, P], fp8, tag="ones_k2")
        eps_t = pp.tile([P, 1], f32, tag="eps")
        nln32 = pp.tile([P, 1], f32, tag="nln32")

        nc.vector.memset(ones_k2, 1.0)
        nc.vector.memset(eps_t, EPS)
        nc.vector.memset(nln32, -3.4657359027997265)

        # ---- input DMAs, first-use order ----
        wqkvT_r = wqkvT_d[:].rearrange("(c p) f -> p c f", p=P)
        xT_r = xT_d[:].rearrange("(c p) t -> p c t", p=P)
        for dcp in range(4):
            nc.scalar.dma_start(
                out=xT[:, 2 * dcp : 2 * dcp + 2, 0:512],
                in_=xT_r[:, 2 * dcp : 2 * dcp + 2, 0:512],
            )
        nc.sync.dma_start(out=wqk[8], in_=wqkvT_r[:, :, 8 * P : 9 * P])
        nc.sync.dma_start(out=wqk[9], in_=wqkvT_r[:, :, 9 * P : 10 * P])
        nc.sync.dma_start(out=wqk[0], in_=wqkvT_r[:, :, 0:P])
        nc.sync.dma_start(out=wqk[1], in_=wqkvT_r[:, :, P : 2 * P])
        for tch in range(1, 4):
            nc.scalar.dma_start(
                out=xT[:, :, tch * 512 : (tch + 1) * 512],
                in_=xT_r[:, :, tch * 512 : (tch + 1) * 512],
            )
        nc.gpsimd.dma_start(out=wv, in_=wqkvT_r[:, :, 2 * D : 3 * D])
        for m in (10, 11, 2, 3, 12, 13, 4, 5, 14, 15, 6, 7):
            nc.sync.dma_start(out=wqk[m], in_=wqkvT_r[:, :, m * P : (m + 1) * P])
        nc.gpsimd.dma_start(out=woT, in_=woT_d[:].rearrange("(c p) o -> p c o", p=P))
        if not ZB:
            nc.gpsimd.dma_start(out=qkvb, in_=qkvb_d[:])
            nc.gpsimd.dma_start(out=vb_bc, in_=vb_d[:])
            nc.sync.dma_start(out=gamma_bc, in_=gamma_d[:])
            nc.sync.dma_start(out=beta_bc, in_=beta_d[:])

        # ---- chunk emitters (each: 4 DoubleRow matmuls + one PSUM->SBUF cast) ----
        def qkv_chunk(m, qc, eng):
            """Q (m<8, 512 local tokens) or K (m>=8, 512 of 2048 tokens)."""
            ps = ps_misc.tile([P, 512], f32, tag="ps_misc")
            for dcp in range(DC // 2):
                nc.tensor.matmul(
                    ps,
                    lhsT=wqk[m][:, 2 * dcp : 2 * dcp + 2, :],
                    rhs=xT[:, 2 * dcp : 2 * dcp + 2, qc * 512 : (qc + 1) * 512],
                    start=(dcp == 0),
                    stop=(dcp == DC // 2 - 1),
                    perf_mode=DR,
                )
            if m < 8:
                dst = qT[:, m, qc * 512 : (qc + 1) * 512]
            else:
                dst = kT[:, m - 8, qc * 512 : (qc + 1) * 512]
            if ZB:
                if eng is nc.scalar:
                    nc.scalar.activation(
                        out=dst, in_=ps, func=AF.Identity, bias=0.0, scale=1.0
                    )
                else:
                    eng.tensor_copy(out=dst, in_=ps)
            else:
                nc.scalar.activation(
                    out=dst, in_=ps, func=AF.Identity, bias=qkvb[:, m : m + 1], scale=1.0
                )

        def v_chunk(vt, oc, eng):
            ps = ps_misc.tile([P, 512], f32, tag="ps_misc")
            for dcp in range(DC // 2):
                nc.tensor.matmul(
                    ps,
                    lhsT=xT[:, 2 * dcp : 2 * dcp + 2, vt * P : (vt + 1) * P],
                    rhs=wv[:, 2 * dcp : 2 * dcp + 2, oc * 512 : (oc + 1) * 512],
                    start=(dcp == 0),
                    stop=(dcp == DC // 2 - 1),
                    perf_mode=DR,
                )
            dst = v[:, vt, oc * 512 : (oc + 1) * 512]
            if ZB:
                if eng is nc.scalar:
                    nc.scalar.activation(
                        out=dst, in_=ps, func=AF.Identity, bias=0.0, scale=1.0
                    )
                else:
                    eng.tensor_copy(out=dst, in_=ps)
            else:
                nc.vector.tensor_add(
                    out=dst, in0=ps, in1=vb_bc[:, oc * 512 : (oc + 1) * 512]
                )

        y_tiles = {}

        def wo_chunk(qt, oc, eng):
            if qt not in y_tiles:
                y_tiles[qt] = pdy.tile([P, D], f32, name=f"y{qt}", tag="y")
            y_t = y_tiles[qt]
            ps = ps_misc.tile([P, 512], f32, tag="ps_misc")
            for dcp in range(DC // 2):
                nc.tensor.matmul(
                    ps,
                    lhsT=outT[:, 2 * dcp : 2 * dcp + 2, qt * P : (qt + 1) * P],
                    rhs=woT[:, 2 * dcp : 2 * dcp + 2, oc * 512 : (oc + 1) * 512],
                    start=(dcp == 0),
                    stop=(dcp == DC // 2 - 1),
                    perf_mode=DR,
                )
            dst = y_t[:, oc * 512 : (oc + 1) * 512]
            if eng is nc.scalar:
                nc.scalar.activation(out=dst, in_=ps, func=AF.Identity, bias=0.0, scale=1.0)
            else:
                eng.tensor_copy(out=dst, in_=ps)

        def ln_tile(qt):
            """residual add + LayerNorm + store for token tile qt."""
            y_t = y_tiles.pop(qt)
            nc.gpsimd.dma_start(
                out=y_t, in_=xb_d[qt * P : (qt + 1) * P, :], accum_op=ALU.add
            )
            stats = pst.tile([P, 2, 6], f32, tag="stats")
            for sg in range(2):
                nc.vector.bn_stats(
                    out=stats[:, sg, :], in_=y_t[:, sg * 512 : (sg + 1) * 512]
                )
            mv = pst.tile([P, 2], f32, tag="mv")
            nc.vector.bn_aggr(out=mv, in_=stats)
            # rstd = exp(-0.5*ln(var+eps)): Ln and Exp share the ACT exp
            # table set, so this never forces a mid-kernel table reload.
            lnv = pst.tile([P, 1], f32, tag="lnv")
            nc.scalar.activation(
                out=lnv, in_=mv[:, 1:2], func=AF.Ln, bias=eps_t, scale=1.0
            )
            rstd = pst.tile([P, 1], f32, tag="rstd")
            nc.scalar.activation(out=rstd, in_=lnv, func=AF.Exp, bias=0.0, scale=-0.5)
            nc.vector.tensor_scalar(
                out=y_t,
                in0=y_t,
                scalar1=mv[:, 0:1],
                scalar2=rstd,
                op0=ALU.subtract,
                op1=ALU.mult,
            )
            if not ZB:
                nc.vector.tensor_mul(out=y_t, in0=y_t, in1=gamma_bc)
                nc.vector.tensor_add(out=y_t, in0=y_t, in1=beta_bc)
            nc.sync.dma_start(out=y_d[qt * P : (qt + 1) * P, :], in_=y_t)

        # ---- softmax finalize (pipelined into the following unit) ----
        def finalize_den(p):
            es_p, ops_p, hqc = p
            dps = ps_misc.tile([P, 512], f32, name="dps", tag="ps_misc")
            for jp in range(KT_TILES // 2):
                nc.tensor.matmul(
                    dps,
                    lhsT=ones_k2,
                    rhs=es_p[:, 2 * jp : 2 * jp + 2, :],
                    start=(jp == 0),
                    stop=(jp == KT_TILES // 2 - 1),
                    perf_mode=DR,
                )
            # 1/den = exp(-ln(den)) on ACT: both funcs live in the exp
            # table set (no reload), and this keeps the DVE free for the
            # projection-chunk casts.  LUT error ~1e-3, diluted by the
            # residual far below the 2e-2 gate.
            lden = pbc.tile([P, 512], f32, tag="lden")
            nc.scalar.activation(out=lden, in_=dps, func=AF.Ln, bias=0.0, scale=1.0)
            bc = pbc.tile([P, 512], f32, tag="bc")
            nc.scalar.activation(out=bc, in_=lden, func=AF.Exp, bias=0.0, scale=-1.0)
            return (ops_p, hqc, bc)

        def finalize_scale(p2):
            ops_p, (qc_p, h_p), bc = p2
            qsl_p = slice(qc_p * 512, (qc_p + 1) * 512)
            for f in range(2):
                nc.vector.tensor_mul(
                    out=outT[:, 2 * h_p + f, qsl_p], in0=ops_p[f], in1=bc
                )

        # ---- the interleaved main loop ----
        state = {"pend": None, "pend2": None}

        def unit(qc, h, carried, start_at=0):
            """One attention unit (512 q tokens x head h), with `carried`
            thunks (projection/wo/LN work) paced across iterations
            [start_at, 16) of its j-loop."""
            n_car = len(carried)
            emitted = 0
            qsl = slice(qc * 512, (qc + 1) * 512)
            es = pes.tile([P, KT_TILES, 512], fp8, tag="es")
            op0 = ps_out.tile([P, 512], f32, tag="ps_out")
            op1 = ps_out.tile([P, 512], f32, tag="ps_out")
            ops = [op0, op1]
            for j in range(KT_TILES):
                sps = ps_sc.tile([P, 512], f32, tag="ps_sc")
                nc.tensor.matmul(
                    sps,
                    lhsT=kT[:, 2 * h : 2 * h + 2, j * P : (j + 1) * P],
                    rhs=qT[:, 2 * h : 2 * h + 2, qsl],
                    start=True,
                    stop=True,
                    perf_mode=DR,
                )
                nc.scalar.activation(
                    out=es[:, j, :],
                    in_=sps,
                    func=AF.Exp,
                    bias=nln32,
                    scale=1.0 / 16.0,
                )
                if j == 4 and state["pend"] is not None:
                    state["pend2"] = finalize_den(state["pend"])
                    state["pend"] = None
                if j == 11 and state["pend2"] is not None:
                    finalize_scale(state["pend2"])
                    state["pend2"] = None
                want = n_car * max(0, j + 1 - start_at) // (KT_TILES - start_at)
                while emitted < want:
                    carried[emitted]()
                    emitted += 1
                if j % 2 == 1:
                    jp = j // 2
                    for f in range(2):
                        nc.tensor.matmul(
                            ops[f],
                            lhsT=v[
                                :,
                                2 * jp : 2 * jp + 2,
                                h * HD + f * P : h * HD + (f + 1) * P,
                            ],
                            rhs=es[:, 2 * jp : 2 * jp + 2, :],
                            start=(jp == 0),
                            stop=(jp == KT_TILES // 2 - 1),
                            perf_mode=DR,
                        )
            while emitted < n_car:
                carried[emitted]()
                emitted += 1
            state["pend"] = (es, ops, (qc, h))

        A, V_ = nc.scalar, nc.vector

        # PRE: everything unit (qc0,h0) needs up front.
        pre = [
            lambda: qkv_chunk(8, 0, A),
            lambda: qkv_chunk(9, 0, V_),
            lambda: qkv_chunk(0, 0, A),
            lambda: qkv_chunk(1, 0, V_),
            lambda: qkv_chunk(8, 1, A),
            lambda: qkv_chunk(9, 1, V_),
            lambda: qkv_chunk(8, 2, A),
            lambda: qkv_chunk(9, 2, V_),
            lambda: qkv_chunk(8, 3, A),
            lambda: qkv_chunk(9, 3, V_),
            lambda: v_chunk(0, 0, A),
            lambda: v_chunk(1, 0, V_),
            lambda: v_chunk(2, 0, A),
            lambda: v_chunk(3, 0, V_),
        ]
        for t in pre:
            t()

        def carry_k(h2):  # K chunks for head h2 (m = 8+2*h2, 9+2*h2), token order
            out = []
            for qc2 in range(4):
                out.append(lambda m=8 + 2 * h2, q2=qc2: qkv_chunk(m, q2, V_))
                out.append(lambda m=9 + 2 * h2, q2=qc2: qkv_chunk(m, q2, V_))
            return out

        def carry_q(h2, qc2):
            return [
                lambda m=2 * h2: qkv_chunk(m, qc2, V_),
                lambda m=2 * h2 + 1: qkv_chunk(m, qc2, V_),
            ]

        def carry_v(vts, oc):
            return [lambda t=t_, o=oc: v_chunk(t, o, V_) for t_ in vts]

        carried_by_unit = [
            # (qc0,h0): rest of v oc0 (self, paced ahead of av) + head1 k/q
            carry_v(range(4, 16), 0) + carry_k(1) + carry_q(1, 0),
            # (qc0,h1): v oc1 first half + head2 k/q
            carry_v(range(0, 8), 1) + carry_k(2) + carry_q(2, 0),
            # (qc0,h2): v oc1 second half (self-paced, needed from jp>=4) + head3 k/q
            carry_v(range(8, 16), 1) + carry_k(3) + carry_q(3, 0),
            # (qc0,h3): all qc1 q chunks
            [t for h2 in range(4) for t in carry_q(h2, 1)],
            # (qc1,h0): nothing until the qc0 finalize lands (j==11); wo qt0
            [lambda: wo_chunk(0, 0, V_), lambda: wo_chunk(0, 1, V_), lambda: ln_tile(0)],
            # (qc1,h1): wo qt1
            [lambda: wo_chunk(1, 0, V_), lambda: wo_chunk(1, 1, V_), lambda: ln_tile(1)],
            # (qc1,h2): wo qt2
            [lambda: wo_chunk(2, 0, V_), lambda: wo_chunk(2, 1, V_), lambda: ln_tile(2)],
            # (qc1,h3): wo qt3
            [lambda: wo_chunk(3, 0, V_), lambda: wo_chunk(3, 1, V_), lambda: ln_tile(3)],
        ]
        units = [(qc, h) for qc in range(NQ) for h in range(H)]
        for (qc, h), carried in zip(units, carried_by_unit, strict=True):
            # (qc1,h0)'s wo work is legal only after the (qc0,h3)
            # finalize_scale, which this unit emits at j==11.
            unit(qc, h, carried, start_at=12 if (qc, h) == (1, 0) else 0)

        # drain: finalize last unit, then the remaining 4 token tiles.
        finalize_scale(finalize_den(state["pend"]))
        for qt in range(4, 8):
            wo_chunk(qt, 0, A)
            wo_chunk(qt, 1, V_)
            ln_tile(qt)

    _split_excess_waits(nc)
    return nc


def _get_nc(ZB):
    key = ("nc", ZB)
    if key not in _CACHE:
        _CACHE[key] = _build(ZB)
    return _CACHE[key]


def kernel(x, qkv_w, qkv_b, wo_w, wo_b, gamma, beta):
    from concourse.bass_utils import run_bass_kernel_spmd

    x = np.asarray(x, dtype=np.float32)
    qkv_w = np.asarray(qkv_w, dtype=np.float32)
    qkv_b = np.asarray(qkv_b, dtype=np.float32)
    wo_w = np.asarray(wo_w, dtype=np.float32)
    wo_b = np.asarray(wo_b, dtype=np.float32)
    gamma = np.asarray(gamma, dtype=np.float32)
    beta = np.asarray(beta, dtype=np.float32)

    wqkvT = np.ascontiguousarray(qkv_w.T).astype(_FP8)
    woT = np.ascontiguousarray(wo_w.T).astype(_FP8)
    qkvb2d = np.ascontiguousarray(qkv_b.reshape(24, P).T)
    vb = np.ascontiguousarray(np.broadcast_to(qkv_b[2 * D :], (P, D))).astype(np.float32)
    gamma_r = np.ascontiguousarray(np.broadcast_to(gamma, (P, D))).astype(np.float32)
    beta_r = np.ascontiguousarray(np.broadcast_to(beta, (P, D))).astype(np.float32)

    in_maps = []
    for c in range(NCORES):
        b, t = divmod(c, 2)
        loc = x[b, t * SL : (t + 1) * SL]  # [SL, D]
        oth = x[b, (1 - t) * SL : (2 - t) * SL]
        xT = np.concatenate([loc.T, oth.T], axis=1).astype(_FP8)  # [D, S]
        xb = (loc + wo_b[None, :]).astype(np.float32)
        in_maps.append(
            {
                "xT": np.ascontiguousarray(xT),
                "wqkvT": wqkvT,
                "woT": woT,
                "xb": xb,
                "qkvb": qkvb2d,
                "vb": vb,
                "gamma": gamma_r,
                "beta": beta_r,
            }
        )

    zb = (
        not qkv_b.any()
        and bool(np.all(gamma == 1.0))
        and not beta.any()
    )
    trace = os.environ.get("BASS_KERNEL_TRACE") == "1"
    res = run_bass_kernel_spmd(
        _get_nc(zb),
        in_maps,
        list(range(NCORES)),
        trace=trace,
        tmpdir=os.environ.get("BASS_KERNEL_TRACE_DIR") or None,
    )
    _CACHE["last_result"] = res

    out = np.empty((B, S, D), dtype=np.float32)
    for c in range(NCORES):
        b, t = divmod(c, 2)
        out[b, t * SL : (t + 1) * SL] = res.results[c]["y"]
    return out
